# revision 6
# baseline (speedup 1.0000x reference)
"""Trainium2 Bass kernel for nn_ActionEncoder (moe_routing).

Math (derived from the reference):
  For sample b with t = action_types[b], i0, i1 = action_indecies[b]:
    type 0: out = tanh(W0[:, i0] + b0)
    type 1: out = tanh(W1[:, i0] + W1[:, 64 + i1] + b1)
  This equals  out = tanh(T0 @ oh0 + T1 @ oh1)  with the reference's 128-wide
  one-hot marks at {i0, 64+i1}, type-masked:
    T0 = [W0 + b0/2 | b0/2 replicated]          (4 x 128)
    T1 = [W1[:, :64] + b1/2 | W1[:, 64:] + b1/2] (4 x 128)
    oh0 = marks if t == 0 else 0 ; oh1 = marks if t == 1 else 0

Device pipeline (pure data parallel, 8 cores x 65536 samples):
  - DMA loads indices interleaved: partition 2j   <- i0 of group j
                                   partition 2j+1 <- i1 of group j
    (group = 512 consecutive samples); types replicated the same way.
  - one fused DVE op per half:  IP = idx + 128*t   (fp16, exact: values < 256)
  - per group (512 samples):
      1 PE matmul "packed broadcast": selector picks partition pair (2j, 2j+1)
        -> psum[128, 512]: rows 0-63 = i0+128t, rows 64-127 = i1+128t
      1 copy psum -> SBUF fp16 (alternating ACT/DVE to balance engines)
      2 DVE is_equal (4x mode, 16-bit):
        oh0 = (raw == iota2),  oh1 = (raw == iota2 + 128)
        where iota2[d] = d & 63  (so rows 0-63 match i0, rows 64-127 match i1;
        the +128t shift makes each compare type-exclusive)
      2 PE table matmuls accumulate z into a shared psum bank
        (4 groups per bank via output col-tiling at partitions 0/32/64/96;
         tables padded to 32 rows so the whole band is written)
      ACT tanh over the full bank -> staging
  - output written feature-major; host reassembles to [B, 4]

TABLE_MODE: "f16"       -> single-pass fp16 tables (~5e-4 rel err, fastest)
            "bf16_hilo" -> bf16 hi + bf16 lo accumulation (~1e-6, 2 extra mms)
"""

import os

import numpy as np

N_CORES = 8
P = 128
TABLE_MODE = os.environ.get("ACTENC_TABLE_MODE", "f16")

_NC_CACHE = {}


def _build_nc(b_core, table_mode):
    import concourse.mybir as mybir
    from concourse import bacc
    from concourse.tile import TileContext

    f32 = mybir.dt.float32
    i32 = mybir.dt.int32
    i16 = mybir.dt.int16
    f16 = mybir.dt.float16 if table_mode == "f16" else mybir.dt.bfloat16
    hilo = table_mode == "bf16_hilo"
    eq = mybir.AluOpType.is_equal

    S = b_core // P
    assert S * P == b_core and S <= 512
    G = 128                # groups of S samples; group j holds samples j*S..
    GH = 64                # groups per "half" (two partitions per group)
    SG, K_, A_ = 4, 8, 4   # psz supergroups x banks x groups-per-bank

    nc = bacc.Bacc("TRN2", target_bir_lowering=False, debug=False)
    idx = nc.dram_tensor("idx", [b_core, 2], i32, kind="ExternalInput")
    typ = nc.dram_tensor("typ", [b_core], i32, kind="ExternalInput")
    ntab = 64 if hilo else 32
    tab0 = nc.dram_tensor("tab0", [P, ntab], f16, kind="ExternalInput")
    tab1 = nc.dram_tensor("tab1", [P, ntab], f16, kind="ExternalInput")
    selq = nc.dram_tensor("selq", [P, 128 * 64], f16, kind="ExternalInput")
    # out[sg, a, o, k, s] = tanh(z)_o of sample (sg*32 + k*4 + a)*S + s
    out = nc.dram_tensor("out", [SG, A_, 4, K_, S], f32, kind="ExternalOutput")

    # pair-contiguous DRAM view: [2 halves, GH groups, 2*S] (4KB rows)
    idxp = idx.rearrange("(h g s) c -> h g (s c)", h=2, s=S)
    typ3 = typ.rearrange("(h g s) -> h g s", h=2, s=S)       # [2, GH, S]

    with TileContext(nc) as tc:
        with tc.tile_pool(name="const", bufs=1) as cpool, \
             tc.tile_pool(name="oh", bufs=3) as ohpool, \
             tc.tile_pool(name="raws", bufs=3) as rpool, \
             tc.tile_pool(name="stage", bufs=2) as spool, \
             tc.tile_pool(name="psb", bufs=3, space="PSUM") as pbpool, \
             tc.tile_pool(name="pszp", bufs=2, space="PSUM") as pzpool:

            # ---- constants ----
            # selector band (host-shipped): view Q[:, 128j : 128j+128]
            # = [e_j x64 | e_{64+j} x64]
            NQ = 128 * 64
            Q = cpool.tile([P, NQ], f16, tag="Q")
            nc.sync.dma_start(out=Q[:, 0:NQ // 2], in_=selq[:, 0:NQ // 2])
            nc.sync.dma_start(out=Q[:, NQ // 2:], in_=selq[:, NQ // 2:])

            # iota2[d] = d mod 64 ; iota2hi = iota2 + 128
            ic = cpool.tile([P, 1], i32, tag="ic")
            nc.gpsimd.iota(ic[0:64, :], pattern=[[1, 1]], base=0,
                           channel_multiplier=1)
            nc.gpsimd.iota(ic[64:128, :], pattern=[[1, 1]], base=0,
                           channel_multiplier=1)
            iota2 = cpool.tile([P, 1], f32, tag="iota2")
            iota2hi = cpool.tile([P, 1], f32, tag="iota2hi")
            nc.vector.tensor_single_scalar(iota2[:], ic[:], 0.0,
                                           mybir.AluOpType.add)
            nc.vector.tensor_single_scalar(iota2hi[:], ic[:], 128.0,
                                           mybir.AluOpType.add)

            T0 = cpool.tile([P, ntab], f16, tag="T0")
            T1 = cpool.tile([P, ntab], f16, tag="T1")
            nc.sync.dma_start(out=T0[:], in_=tab0[:])
            nc.sync.dma_start(out=T1[:], in_=tab1[:])

            # ---- load index pairs (contiguous 4KB rows) duplicated to both
            #      partition halves; types likewise.  Chunked by 32 groups so
            #      early supergroups start before all input has landed. ----
            IPraw = [cpool.tile([P, 2 * S], i32, tag=f"IPraw{h}", name=f"IPraw{h}") for h in range(2)]
            IPT = [cpool.tile([P, S], i32, tag=f"IPT{h}", name=f"IPT{h}") for h in range(2)]
            IP = [cpool.tile([P, S], f16, tag=f"IP{h}", name=f"IP{h}") for h in range(2)]
            for h in range(2):
                prw3 = IPraw[h][:].rearrange("p (s c) -> p c s", c=2)
                for jc in range(2):
                    gsl = slice(32 * jc, 32 * jc + 32)
                    for c in range(2):
                        rsl = slice(64 * c + 32 * jc, 64 * c + 32 * jc + 32)
                        nc.sync.dma_start(out=IPraw[h][rsl, :],
                                          in_=idxp[h, gsl])
                        nc.sync.dma_start(out=IPT[h][rsl, :],
                                          in_=typ3[h, gsl])
                        nc.vector.scalar_tensor_tensor(
                            out=IP[h][rsl, :],
                            in0=IPT[h][rsl, :], scalar=128.0,
                            in1=prw3[rsl, c, :],
                            op0=mybir.AluOpType.mult,
                            op1=mybir.AluOpType.add)

            # ---- main loop: quads of 4 groups; psb double-buffered pairs ----
            for sg in range(SG):
                stage = spool.tile([P, K_ * S], f32, tag="stage")
                for k in range(K_):
                    psz = pzpool.tile([P, S], f32, tag="psz")
                    raw = rpool.tile([P, 4 * S], f16, tag="raw", name="raw")
                    for half in range(2):
                        pi = (sg * K_ + k) * 2 + half
                        # 2 broadcast matmuls into a 2-bank psum pair
                        psb = pbpool.tile([P, 2 * S], f32, tag="psb",
                                          name="psb")
                        for i in range(2):
                            a = half * 2 + i
                            g = sg * 32 + k * 4 + a
                            h, j = divmod(g, GH)
                            sel = Q[:, 128 * j:128 * j + 128]
                            nc.tensor.matmul(psb[:, i * S:(i + 1) * S],
                                             lhsT=sel, rhs=IP[h][:],
                                             start=True, stop=True)
                        # batched copy psum->sbuf fp16 (alternate ACT/DVE)
                        rsl = slice(half * 2 * S, half * 2 * S + 2 * S)
                        if pi % 4 == 3:
                            nc.vector.tensor_copy(out=raw[:, rsl], in_=psb[:])
                        else:
                            nc.scalar.copy(raw[:, rsl], psb[:])
                    # two quad-wide compares -> both type-masked one-hots
                    oh0 = ohpool.tile([P, 4 * S], f16, tag="oh0", name="oh0")
                    oh1 = ohpool.tile([P, 4 * S], f16, tag="oh1", name="oh1")
                    nc.vector.tensor_single_scalar(oh0[:], raw[:],
                                                   iota2[:], eq)
                    nc.vector.tensor_single_scalar(oh1[:], raw[:],
                                                   iota2hi[:], eq)
                    # 8 gather matmuls back-to-back accumulate z
                    for i2 in range(2):
                        for a in range(A_):
                            ohx = oh0 if i2 == 0 else oh1
                            tab = T0 if i2 == 0 else T1
                            pz = psz[32 * a:32 * a + 32, :]
                            tp = (0, 32 * a)
                            sl = slice(a * S, a * S + S)
                            if hilo:
                                nc.tensor.matmul(pz, lhsT=tab[:, 0:32],
                                                 rhs=ohx[:, sl],
                                                 start=(i2 == 0), stop=False,
                                                 tile_position=tp,
                                                 skip_group_check=True)
                                nc.tensor.matmul(pz, lhsT=tab[:, 32:64],
                                                 rhs=ohx[:, sl],
                                                 start=False, stop=(i2 == 1),
                                                 tile_position=tp,
                                                 skip_group_check=True)
                            else:
                                nc.tensor.matmul(pz, lhsT=tab[:],
                                                 rhs=ohx[:, sl],
                                                 start=(i2 == 0),
                                                 stop=(i2 == 1),
                                                 tile_position=tp,
                                                 skip_group_check=True)
                    nc.scalar.activation(
                        out=stage[:, k * S:(k + 1) * S], in_=psz[:],
                        func=mybir.ActivationFunctionType.Tanh)
                for a in range(A_):
                    src = stage[32 * a:32 * a + 4, :].rearrange(
                        "p (k s) -> p k s", s=S)
                    nc.sync.dma_start(out=out[sg, a], in_=src)

    nc.compile()
    return nc, (SG, A_, K_, S)


def _selq(table_mode):
    import ml_dtypes
    dt = np.float16 if table_mode == "f16" else ml_dtypes.bfloat16
    Q = np.zeros((128, 128 * 64), dt)
    k = np.arange(64)
    f = np.arange(128 * 64)
    top = ((f[None, :] - 128 * k[:, None]) >= 0) & \
          ((f[None, :] - 128 * k[:, None]) < 64)
    bot = ((f[None, :] - 128 * k[:, None] - 64) >= 0) & \
          ((f[None, :] - 128 * k[:, None] - 64) < 128 - 64)
    Q[0:64, :] = top.astype(dt)
    Q[64:128, :] = bot.astype(dt)
    return Q


def _tables(W0, b0, W1, b1, table_mode):
    import ml_dtypes
    W0 = np.asarray(W0, np.float32)
    W1 = np.asarray(W1, np.float32)
    b0 = np.asarray(b0, np.float32).reshape(-1)
    b1 = np.asarray(b1, np.float32).reshape(-1)
    # full-precision tables [128, 4] (row = one-hot position, col = feature)
    T0 = np.concatenate([W0.T + b0 / 2, np.tile(b0 / 2, (64, 1))], axis=0)
    T1 = np.concatenate([W1[:, :64].T + b1 / 2, W1[:, 64:].T + b1 / 2], axis=0)
    if table_mode == "f16":
        dt = ml_dtypes.float16 if hasattr(ml_dtypes, "float16") else np.float16
        out0 = np.zeros((128, 32), np.float16)
        out1 = np.zeros((128, 32), np.float16)
        out0[:, :4] = T0.astype(np.float16)
        out1[:, :4] = T1.astype(np.float16)
        return out0, out1
    else:
        bf = ml_dtypes.bfloat16
        out0 = np.zeros((128, 64), bf)
        out1 = np.zeros((128, 64), bf)
        hi0 = T0.astype(bf)
        hi1 = T1.astype(bf)
        out0[:, 0:4] = hi0
        out0[:, 32:36] = (T0 - hi0.astype(np.float32)).astype(bf)
        out1[:, 0:4] = hi1
        out1[:, 32:36] = (T1 - hi1.astype(np.float32)).astype(bf)
        return out0, out1


def kernel(action_indecies, action_n_obj, action_types, W0, b0, W1, b1,
           **_unused):
    from concourse.bass_utils import run_bass_kernel_spmd

    idx = np.ascontiguousarray(np.asarray(action_indecies, dtype=np.int32))
    typ = np.ascontiguousarray(np.asarray(action_types, dtype=np.int32))
    B = idx.shape[0]
    b_core = B // N_CORES
    assert b_core * N_CORES == B

    tab0, tab1 = _tables(W0, b0, W1, b1, TABLE_MODE)
    selq = _selq(TABLE_MODE)

    key = (b_core, TABLE_MODE)
    if key not in _NC_CACHE:
        _NC_CACHE[key] = _build_nc(b_core, TABLE_MODE)
    nc, (SG, A_, K_, S) = _NC_CACHE[key]

    in_maps = [
        {"idx": idx[k * b_core:(k + 1) * b_core],
         "typ": typ[k * b_core:(k + 1) * b_core],
         "tab0": tab0, "tab1": tab1, "selq": selq}
        for k in range(N_CORES)
    ]
    res = run_bass_kernel_spmd(nc, in_maps, core_ids=list(range(N_CORES)))

    outs = []
    for r in res.results:
        o5 = r["out"]  # [SG, A, 4, K, S]
        o = np.transpose(o5, (0, 3, 1, 4, 2)).reshape(b_core, 4)
        outs.append(o)
    return np.ascontiguousarray(np.concatenate(outs, axis=0))



# revision 12
# speedup vs baseline: 1.0768x; 1.0768x over previous
"""Trainium2 Bass kernel for nn_ActionEncoder (moe_routing).

Math (derived from the reference):
  For sample b with t = action_types[b], i0, i1 = action_indecies[b]:
    type 0: out = tanh(W0[:, i0] + b0)
    type 1: out = tanh(W1[:, i0] + W1[:, 64 + i1] + b1)
  This equals  out = tanh(T0 @ oh0 + T1 @ oh1)  with the reference's 128-wide
  one-hot marks at {i0, 64+i1}, type-masked:
    T0 = [W0 + b0/2 | b0/2 replicated]          (4 x 128)
    T1 = [W1[:, :64] + b1/2 | W1[:, 64:] + b1/2] (4 x 128)
    oh0 = marks if t == 0 else 0 ; oh1 = marks if t == 1 else 0

Device pipeline (pure data parallel, 8 cores x 65536 samples):
  - DMA loads indices interleaved: partition 2j   <- i0 of group j
                                   partition 2j+1 <- i1 of group j
    (group = 512 consecutive samples); types replicated the same way.
  - one fused DVE op per half:  IP = idx + 128*t   (fp16, exact: values < 256)
  - per group (512 samples):
      1 PE matmul "packed broadcast": selector picks partition pair (2j, 2j+1)
        -> psum[128, 512]: rows 0-63 = i0+128t, rows 64-127 = i1+128t
      copy psum -> SBUF fp16 (rotating ACT/DVE to balance engines)
      2 DVE is_equal (4x mode, 16-bit):
        oh0 = (raw == iota2),  oh1 = (raw == iota2 + 128)
        (some oh1 compares run on the otherwise-idle GPSIMD engine)
  - gather matmuls accumulate 16 groups (4 quads) into ONE psum bank:
    band a (32 rows) collects quad q's 4 outputs at rows 32a + 8q + o via
    column-shifted table variants (zero cols elsewhere accumulate 0).
  - one tanh per bank [128, S] -> fp16 stage, one output DMA per bank
  - output written feature-major fp16; host reassembles to [B, 4] fp32
"""

import os

import numpy as np

N_CORES = 8
P = 128
GPS_CMP = int(os.environ.get("ACTENC_GPS_CMP", "0"))
DVE_COPY_MOD = int(os.environ.get("ACTENC_DVE_COPY_MOD", "4"))

_NC_CACHE = {}


def _build_nc(b_core):
    import concourse.mybir as mybir
    from concourse import bacc
    from concourse.tile import TileContext

    f32 = mybir.dt.float32
    i32 = mybir.dt.int32
    f16 = mybir.dt.float16
    eq = mybir.AluOpType.is_equal

    S = b_core // P
    assert S * P == b_core and S <= 512
    G = 128                # groups of S samples; group j holds samples j*S..
    GH = 64                # groups per "half" (two partitions per group)
    NB, QD, A_ = 8, 4, 4   # psum bank-groups x quads-per-bank x groups-per-quad

    nc = bacc.Bacc("TRN2", target_bir_lowering=False, debug=False)
    idx = nc.dram_tensor("idx", [b_core, 2], i32, kind="ExternalInput")
    typ = nc.dram_tensor("typ", [b_core], i32, kind="ExternalInput")
    # 4 column-shifted variants per table: variant q lives in cols 32q..32q+31
    # with the real 4 feature columns at local offset 4q (rest zero).
    tab0 = nc.dram_tensor("tab0", [P, 128], f16, kind="ExternalInput")
    tab1 = nc.dram_tensor("tab1", [P, 128], f16, kind="ExternalInput")
    selq = nc.dram_tensor("selq", [P, 128 * 64], f16, kind="ExternalInput")
    # out[nb, a, q, o, s] = tanh(z)_o of sample (nb*16 + q*4 + a)*S + s
    out = nc.dram_tensor("out", [NB, A_, QD * 4, S], f16, kind="ExternalOutput")

    # pair-contiguous DRAM view: [2 halves, GH groups, 2*S] (4KB rows)
    idxp = idx.rearrange("(h g s) c -> h g (s c)", h=2, s=S)
    typ3 = typ.rearrange("(h g s) -> h g s", h=2, s=S)       # [2, GH, S]

    with TileContext(nc) as tc:
        with tc.tile_pool(name="const", bufs=1) as cpool, \
             tc.tile_pool(name="oh", bufs=4) as ohpool, \
             tc.tile_pool(name="raws", bufs=4) as rpool, \
             tc.tile_pool(name="stage", bufs=2) as spool, \
             tc.tile_pool(name="psb", bufs=3, space="PSUM") as pbpool, \
             tc.tile_pool(name="pszp", bufs=2, space="PSUM") as pzpool:

            # ---- constants ----
            # selector band (host-shipped): view Q[:, 128j : 128j+128]
            # = [e_j x64 | e_{64+j} x64]
            NQ = 128 * 64
            Q = cpool.tile([P, NQ], f16, tag="Q")
            nc.sync.dma_start(out=Q[:, 0:NQ // 2], in_=selq[:, 0:NQ // 2])
            nc.sync.dma_start(out=Q[:, NQ // 2:], in_=selq[:, NQ // 2:])

            # iota2[d] = d mod 64 ; iota2hi = iota2 + 128
            ic = cpool.tile([P, 1], i32, tag="ic")
            nc.gpsimd.iota(ic[0:64, :], pattern=[[1, 1]], base=0,
                           channel_multiplier=1)
            nc.gpsimd.iota(ic[64:128, :], pattern=[[1, 1]], base=0,
                           channel_multiplier=1)
            iota2 = cpool.tile([P, 1], f32, tag="iota2")
            iota2hi = cpool.tile([P, 1], f32, tag="iota2hi")
            nc.vector.tensor_single_scalar(iota2[:], ic[:], 0.0,
                                           mybir.AluOpType.add)
            nc.vector.tensor_single_scalar(iota2hi[:], ic[:], 128.0,
                                           mybir.AluOpType.add)

            T0 = cpool.tile([P, 128], f16, tag="T0")
            T1 = cpool.tile([P, 128], f16, tag="T1")
            nc.sync.dma_start(out=T0[:], in_=tab0[:])
            nc.sync.dma_start(out=T1[:], in_=tab1[:])

            # ---- load index pairs (contiguous 4KB rows) duplicated to both
            #      partition halves; types likewise.  Chunked by 32 groups so
            #      early bank-groups start before all input has landed. ----
            IPraw = [cpool.tile([P, 2 * S], i32, tag=f"IPraw{h}",
                                name=f"IPraw{h}") for h in range(2)]
            IPT = [cpool.tile([P, S], i32, tag=f"IPT{h}", name=f"IPT{h}")
                   for h in range(2)]
            IP = [cpool.tile([P, S], f16, tag=f"IP{h}", name=f"IP{h}")
                  for h in range(2)]
            for h in range(2):
                prw3 = IPraw[h][:].rearrange("p (s c) -> p c s", c=2)
                for jc in range(2):
                    gsl = slice(32 * jc, 32 * jc + 32)
                    for c in range(2):
                        rsl = slice(64 * c + 32 * jc, 64 * c + 32 * jc + 32)
                        nc.sync.dma_start(out=IPraw[h][rsl, :],
                                          in_=idxp[h, gsl])
                        nc.sync.dma_start(out=IPT[h][rsl, :],
                                          in_=typ3[h, gsl])
                        nc.vector.scalar_tensor_tensor(
                            out=IP[h][rsl, :],
                            in0=IPT[h][rsl, :], scalar=128.0,
                            in1=prw3[rsl, c, :],
                            op0=mybir.AluOpType.mult,
                            op1=mybir.AluOpType.add)

            # ---- main loop: 8 bank-groups of 4 quads x 4 groups ----
            for nb in range(NB):
                psz = pzpool.tile([P, S], f32, tag="psz")
                stage = spool.tile([P, S], f16, tag="stage")
                for q in range(QD):
                    pi = nb * QD + q
                    raw = rpool.tile([P, 4 * S], f16, tag="raw", name="raw")
                    for half in range(2):
                        # 2 broadcast matmuls into a 2-bank psum pair
                        psb = pbpool.tile([P, 2 * S], f32, tag="psb",
                                          name="psb")
                        for i in range(2):
                            a = half * 2 + i
                            g = nb * 16 + q * 4 + a
                            h, j = divmod(g, GH)
                            sel = Q[:, 128 * j:128 * j + 128]
                            nc.tensor.matmul(psb[:, i * S:(i + 1) * S],
                                             lhsT=sel, rhs=IP[h][:],
                                             start=True, stop=True)
                        # batched copy psum->sbuf fp16 (rotate ACT/DVE)
                        rsl = slice(half * 2 * S, half * 2 * S + 2 * S)
                        ci = pi * 2 + half
                        if ci % DVE_COPY_MOD == DVE_COPY_MOD - 1:
                            nc.vector.tensor_copy(out=raw[:, rsl], in_=psb[:])
                        else:
                            nc.scalar.copy(raw[:, rsl], psb[:])
                    # two quad-wide compares -> both type-masked one-hots
                    oh0 = ohpool.tile([P, 4 * S], f16, tag="oh0", name="oh0")
                    oh1 = ohpool.tile([P, 4 * S], f16, tag="oh1", name="oh1")
                    nc.vector.tensor_single_scalar(oh0[:], raw[:],
                                                   iota2[:], eq)
                    if GPS_CMP and pi % 2 == 0:
                        nc.gpsimd.tensor_single_scalar(oh1[:], raw[:],
                                                       iota2hi[:], eq)
                    else:
                        nc.vector.tensor_single_scalar(oh1[:], raw[:],
                                                       iota2hi[:], eq)
                    # 8 gather matmuls accumulate into the 4-quad-packed bank
                    for i2 in range(2):
                        ohx = oh0 if i2 == 0 else oh1
                        tab = T0 if i2 == 0 else T1
                        for a in range(A_):
                            pz = psz[32 * a:32 * a + 32, :]
                            nc.tensor.matmul(
                                pz, lhsT=tab[:, 32 * q:32 * q + 32],
                                rhs=ohx[:, a * S:a * S + S],
                                start=(q == 0 and i2 == 0),
                                stop=(q == QD - 1 and i2 == 1),
                                tile_position=(0, 32 * a),
                                skip_group_check=True)
                # one tanh per bank [128, S]: rows 32a+4q+o hold group
                # (nb*16 + q*4 + a), feature o
                nc.scalar.activation(out=stage[:], in_=psz[:],
                                     func=mybir.ActivationFunctionType.Tanh)
                for a in range(A_):
                    nc.sync.dma_start(out=out[nb, a],
                                      in_=stage[32 * a:32 * a + 16, :])

    nc.compile()
    return nc, (NB, A_, QD, S)


def _selq():
    Q = np.zeros((128, 128 * 64), np.float16)
    k = np.arange(64)
    f = np.arange(128 * 64)
    top = ((f[None, :] - 128 * k[:, None]) >= 0) & \
          ((f[None, :] - 128 * k[:, None]) < 64)
    bot = ((f[None, :] - 128 * k[:, None] - 64) >= 0) & \
          ((f[None, :] - 128 * k[:, None] - 64) < 128 - 64)
    Q[0:64, :] = top.astype(np.float16)
    Q[64:128, :] = bot.astype(np.float16)
    return Q


def _tables(W0, b0, W1, b1):
    W0 = np.asarray(W0, np.float32)
    W1 = np.asarray(W1, np.float32)
    b0 = np.asarray(b0, np.float32).reshape(-1)
    b1 = np.asarray(b1, np.float32).reshape(-1)
    # full-precision tables [128, 4] (row = one-hot position, col = feature)
    T0 = np.concatenate([W0.T + b0 / 2, np.tile(b0 / 2, (64, 1))], axis=0)
    T1 = np.concatenate([W1[:, :64].T + b1 / 2, W1[:, 64:].T + b1 / 2], axis=0)
    # 4 column-shifted variants: variant q in cols 32q..32q+31, features at
    # local cols 4q..4q+3 (so quad q's outputs land at psum rows 32a+4q+o)
    out0 = np.zeros((128, 128), np.float16)
    out1 = np.zeros((128, 128), np.float16)
    for q in range(4):
        out0[:, 32 * q + 4 * q:32 * q + 4 * q + 4] = T0.astype(np.float16)
        out1[:, 32 * q + 4 * q:32 * q + 4 * q + 4] = T1.astype(np.float16)
    return out0, out1


def kernel(action_indecies, action_n_obj, action_types, W0, b0, W1, b1,
           **_unused):
    from concourse.bass_utils import run_bass_kernel_spmd

    idx = np.ascontiguousarray(np.asarray(action_indecies, dtype=np.int32))
    typ = np.ascontiguousarray(np.asarray(action_types, dtype=np.int32))
    B = idx.shape[0]
    b_core = B // N_CORES
    assert b_core * N_CORES == B

    tab0, tab1 = _tables(W0, b0, W1, b1)
    selq = _selq()

    key = b_core
    if key not in _NC_CACHE:
        _NC_CACHE[key] = _build_nc(b_core)
    nc, (NB, A_, QD, S) = _NC_CACHE[key]

    in_maps = [
        {"idx": idx[k * b_core:(k + 1) * b_core],
         "typ": typ[k * b_core:(k + 1) * b_core],
         "tab0": tab0, "tab1": tab1, "selq": selq}
        for k in range(N_CORES)
    ]
    res = run_bass_kernel_spmd(nc, in_maps, core_ids=list(range(N_CORES)))

    outs = []
    for r in res.results:
        o4 = r["out"]  # [NB, A, 16, S]; row 4q+o = feature o of group nb*16+q*4+a
        o5 = o4.reshape(NB, A_, QD, 4, S)
        o = np.transpose(o5, (0, 2, 1, 4, 3)).reshape(b_core, 4)
        outs.append(o)
    return np.ascontiguousarray(np.concatenate(outs, axis=0).astype(np.float32))


# revision 14
# speedup vs baseline: 1.1159x; 1.0363x over previous
"""Trainium2 Bass kernel for nn_ActionEncoder (moe_routing).

Math (derived from the reference):
  For sample b with t = action_types[b], i0, i1 = action_indecies[b]:
    type 0: out = tanh(W0[:, i0] + b0)
    type 1: out = tanh(W1[:, i0] + W1[:, 64 + i1] + b1)
  This equals  out = tanh(T0 @ oh0 + T1 @ oh1)  with the reference's 128-wide
  one-hot marks at {i0, 64+i1}, type-masked:
    T0 = [W0 + b0/2 | b0/2 replicated]          (4 x 128)
    T1 = [W1[:, :64] + b1/2 | W1[:, 64:] + b1/2] (4 x 128)
    oh0 = marks if t == 0 else 0 ; oh1 = marks if t == 1 else 0

Device pipeline (pure data parallel, 8 cores x 65536 samples):
  - DMA loads indices interleaved: partition 2j   <- i0 of group j
                                   partition 2j+1 <- i1 of group j
    (group = 512 consecutive samples); types replicated the same way.
  - one fused DVE op per half:  IP = idx + 128*t   (fp16, exact: values < 256)
  - per group (512 samples):
      1 PE matmul "packed broadcast": selector picks partition pair (2j, 2j+1)
        -> psum[128, 512]: rows 0-63 = i0+128t, rows 64-127 = i1+128t
      copy psum -> SBUF fp16 (rotating ACT/DVE to balance engines)
      2 DVE is_equal (4x mode, 16-bit):
        oh0 = (raw == iota2),  oh1 = (raw == iota2 + 128)
        (some oh1 compares run on the otherwise-idle GPSIMD engine)
  - gather matmuls accumulate 16 groups (4 quads) into ONE psum bank:
    band a (32 rows) collects quad q's 4 outputs at rows 32a + 8q + o via
    column-shifted table variants (zero cols elsewhere accumulate 0).
  - one tanh per bank [128, S] -> fp16 stage, one output DMA per bank
  - output written feature-major fp16; host reassembles to [B, 4] fp32
"""

import os

import numpy as np

N_CORES = 8
P = 128
GPS_CMP = int(os.environ.get("ACTENC_GPS_CMP", "0"))
DVE_COPY_MOD = int(os.environ.get("ACTENC_DVE_COPY_MOD", "7"))

_NC_CACHE = {}


def _build_nc(b_core):
    import concourse.mybir as mybir
    from concourse import bacc
    from concourse.tile import TileContext

    f32 = mybir.dt.float32
    i32 = mybir.dt.int32
    f16 = mybir.dt.float16
    eq = mybir.AluOpType.is_equal

    S = b_core // P
    assert S * P == b_core and S <= 512
    G = 128                # groups of S samples; group j holds samples j*S..
    GH = 64                # groups per "half" (two partitions per group)
    NB, QD, A_ = 8, 4, 4   # psum bank-groups x quads-per-bank x groups-per-quad

    nc = bacc.Bacc("TRN2", target_bir_lowering=False, debug=False)
    idx = nc.dram_tensor("idx", [b_core, 2], i32, kind="ExternalInput")
    typ = nc.dram_tensor("typ", [b_core], i32, kind="ExternalInput")
    # 4 column-shifted variants per table: variant q lives in cols 32q..32q+31
    # with the real 4 feature columns at local offset 4q (rest zero).
    tab0 = nc.dram_tensor("tab0", [P, 128], f16, kind="ExternalInput")
    tab1 = nc.dram_tensor("tab1", [P, 128], f16, kind="ExternalInput")
    selq = nc.dram_tensor("selq", [P, 128 * 64], f16, kind="ExternalInput")
    # out[nb, a, q, o, s] = tanh(z)_o of sample (nb*16 + q*4 + a)*S + s
    out = nc.dram_tensor("out", [NB, A_, QD * 4, S], f16, kind="ExternalOutput")

    # pair-contiguous DRAM view: [2 halves, GH groups, 2*S] (4KB rows)
    idxp = idx.rearrange("(h g s) c -> h g (s c)", h=2, s=S)
    typ3 = typ.rearrange("(h g s) -> h g s", h=2, s=S)       # [2, GH, S]

    with TileContext(nc) as tc:
        with tc.tile_pool(name="const", bufs=1) as cpool, \
             tc.tile_pool(name="oh", bufs=4) as ohpool, \
             tc.tile_pool(name="raws", bufs=4) as rpool, \
             tc.tile_pool(name="stage", bufs=2) as spool, \
             tc.tile_pool(name="psb", bufs=3, space="PSUM") as pbpool, \
             tc.tile_pool(name="pszp", bufs=2, space="PSUM") as pzpool:

            # ---- constants ----
            # selector band (host-shipped): view Q[:, 128j : 128j+128]
            # = [e_j x64 | e_{64+j} x64]
            NQ = 128 * 64
            Q = cpool.tile([P, NQ], f16, tag="Q")
            nc.sync.dma_start(out=Q[:, 0:NQ // 2], in_=selq[:, 0:NQ // 2])
            nc.sync.dma_start(out=Q[:, NQ // 2:], in_=selq[:, NQ // 2:])

            # iota2[d] = d mod 64 ; iota2hi = iota2 + 128
            ic = cpool.tile([P, 1], i32, tag="ic")
            nc.gpsimd.iota(ic[0:64, :], pattern=[[1, 1]], base=0,
                           channel_multiplier=1)
            nc.gpsimd.iota(ic[64:128, :], pattern=[[1, 1]], base=0,
                           channel_multiplier=1)
            iota2 = cpool.tile([P, 1], f32, tag="iota2")
            iota2hi = cpool.tile([P, 1], f32, tag="iota2hi")
            nc.vector.tensor_single_scalar(iota2[:], ic[:], 0.0,
                                           mybir.AluOpType.add)
            nc.vector.tensor_single_scalar(iota2hi[:], ic[:], 128.0,
                                           mybir.AluOpType.add)

            T0 = cpool.tile([P, 128], f16, tag="T0")
            T1 = cpool.tile([P, 128], f16, tag="T1")
            nc.sync.dma_start(out=T0[:], in_=tab0[:])
            nc.sync.dma_start(out=T1[:], in_=tab1[:])

            # ---- load index pairs (contiguous 4KB rows) duplicated to both
            #      partition halves; types likewise.  Chunked by 32 groups so
            #      early bank-groups start before all input has landed. ----
            IPraw = [cpool.tile([P, 2 * S], i32, tag=f"IPraw{h}",
                                name=f"IPraw{h}") for h in range(2)]
            IPT = [cpool.tile([P, S], i32, tag=f"IPT{h}", name=f"IPT{h}")
                   for h in range(2)]
            IP = [cpool.tile([P, S], f16, tag=f"IP{h}", name=f"IP{h}")
                  for h in range(2)]
            for h in range(2):
                prw3 = IPraw[h][:].rearrange("p (s c) -> p c s", c=2)
                for jc in range(2):
                    gsl = slice(32 * jc, 32 * jc + 32)
                    for c in range(2):
                        rsl = slice(64 * c + 32 * jc, 64 * c + 32 * jc + 32)
                        nc.sync.dma_start(out=IPraw[h][rsl, :],
                                          in_=idxp[h, gsl])
                        nc.sync.dma_start(out=IPT[h][rsl, :],
                                          in_=typ3[h, gsl])
                        nc.vector.scalar_tensor_tensor(
                            out=IP[h][rsl, :],
                            in0=IPT[h][rsl, :], scalar=128.0,
                            in1=prw3[rsl, c, :],
                            op0=mybir.AluOpType.mult,
                            op1=mybir.AluOpType.add)

            # ---- main loop: 8 bank-groups of 4 quads x 4 groups,
            #      software-pipelined so quad pi+1's broadcasts+copies issue
            #      on PE/ACT before quad pi's gathers (in-order engines would
            #      otherwise stall next-quad work behind compare-blocked
            #      gathers) ----
            NPI = NB * QD

            def emit_spread(pi):
                raw = rpool.tile([P, 4 * S], f16, tag="raw", name="raw")
                for half in range(2):
                    # 2 broadcast matmuls into a 2-bank psum pair
                    psb = pbpool.tile([P, 2 * S], f32, tag="psb", name="psb")
                    for i in range(2):
                        g = pi * 4 + half * 2 + i
                        h, j = divmod(g, GH)
                        sel = Q[:, 128 * j:128 * j + 128]
                        nc.tensor.matmul(psb[:, i * S:(i + 1) * S],
                                         lhsT=sel, rhs=IP[h][:],
                                         start=True, stop=True)
                    # batched copy psum->sbuf fp16 (rotate ACT/DVE)
                    rsl = slice(half * 2 * S, half * 2 * S + 2 * S)
                    ci = pi * 2 + half
                    if ci % DVE_COPY_MOD == DVE_COPY_MOD - 1:
                        nc.vector.tensor_copy(out=raw[:, rsl], in_=psb[:])
                    else:
                        nc.scalar.copy(raw[:, rsl], psb[:])
                return raw

            psz = None
            raw_next = emit_spread(0)
            for pi in range(NPI):
                nb, q = divmod(pi, QD)
                if q == 0:
                    psz = pzpool.tile([P, S], f32, tag="psz")
                raw = raw_next
                # two quad-wide compares -> both type-masked one-hots
                oh0 = ohpool.tile([P, 4 * S], f16, tag="oh0", name="oh0")
                oh1 = ohpool.tile([P, 4 * S], f16, tag="oh1", name="oh1")
                nc.vector.tensor_single_scalar(oh0[:], raw[:], iota2[:], eq)
                nc.vector.tensor_single_scalar(oh1[:], raw[:], iota2hi[:], eq)
                if pi + 1 < NPI:
                    raw_next = emit_spread(pi + 1)
                # 8 gather matmuls accumulate into the 4-quad-packed bank
                for i2 in range(2):
                    ohx = oh0 if i2 == 0 else oh1
                    tab = T0 if i2 == 0 else T1
                    for a in range(A_):
                        pz = psz[32 * a:32 * a + 32, :]
                        nc.tensor.matmul(
                            pz, lhsT=tab[:, 32 * q:32 * q + 32],
                            rhs=ohx[:, a * S:a * S + S],
                            start=(q == 0 and i2 == 0),
                            stop=(q == QD - 1 and i2 == 1),
                            tile_position=(0, 32 * a),
                            skip_group_check=True)
                if q == QD - 1:
                    # one tanh per bank [128, S]: rows 32a+4q+o hold group
                    # (nb*16 + q*4 + a), feature o
                    stage = spool.tile([P, S], f16, tag="stage")
                    nc.scalar.activation(
                        out=stage[:], in_=psz[:],
                        func=mybir.ActivationFunctionType.Tanh)
                    for a in range(A_):
                        nc.sync.dma_start(out=out[nb, a],
                                          in_=stage[32 * a:32 * a + 16, :])

    nc.compile()
    return nc, (NB, A_, QD, S)


def _selq():
    Q = np.zeros((128, 128 * 64), np.float16)
    k = np.arange(64)
    f = np.arange(128 * 64)
    top = ((f[None, :] - 128 * k[:, None]) >= 0) & \
          ((f[None, :] - 128 * k[:, None]) < 64)
    bot = ((f[None, :] - 128 * k[:, None] - 64) >= 0) & \
          ((f[None, :] - 128 * k[:, None] - 64) < 128 - 64)
    Q[0:64, :] = top.astype(np.float16)
    Q[64:128, :] = bot.astype(np.float16)
    return Q


def _tables(W0, b0, W1, b1):
    W0 = np.asarray(W0, np.float32)
    W1 = np.asarray(W1, np.float32)
    b0 = np.asarray(b0, np.float32).reshape(-1)
    b1 = np.asarray(b1, np.float32).reshape(-1)
    # full-precision tables [128, 4] (row = one-hot position, col = feature)
    T0 = np.concatenate([W0.T + b0 / 2, np.tile(b0 / 2, (64, 1))], axis=0)
    T1 = np.concatenate([W1[:, :64].T + b1 / 2, W1[:, 64:].T + b1 / 2], axis=0)
    # 4 column-shifted variants: variant q in cols 32q..32q+31, features at
    # local cols 4q..4q+3 (so quad q's outputs land at psum rows 32a+4q+o)
    out0 = np.zeros((128, 128), np.float16)
    out1 = np.zeros((128, 128), np.float16)
    for q in range(4):
        out0[:, 32 * q + 4 * q:32 * q + 4 * q + 4] = T0.astype(np.float16)
        out1[:, 32 * q + 4 * q:32 * q + 4 * q + 4] = T1.astype(np.float16)
    return out0, out1


def kernel(action_indecies, action_n_obj, action_types, W0, b0, W1, b1,
           **_unused):
    from concourse.bass_utils import run_bass_kernel_spmd

    idx = np.ascontiguousarray(np.asarray(action_indecies, dtype=np.int32))
    typ = np.ascontiguousarray(np.asarray(action_types, dtype=np.int32))
    B = idx.shape[0]
    b_core = B // N_CORES
    assert b_core * N_CORES == B

    tab0, tab1 = _tables(W0, b0, W1, b1)
    selq = _selq()

    key = b_core
    if key not in _NC_CACHE:
        _NC_CACHE[key] = _build_nc(b_core)
    nc, (NB, A_, QD, S) = _NC_CACHE[key]

    in_maps = [
        {"idx": idx[k * b_core:(k + 1) * b_core],
         "typ": typ[k * b_core:(k + 1) * b_core],
         "tab0": tab0, "tab1": tab1, "selq": selq}
        for k in range(N_CORES)
    ]
    res = run_bass_kernel_spmd(nc, in_maps, core_ids=list(range(N_CORES)))

    outs = []
    for r in res.results:
        o4 = r["out"]  # [NB, A, 16, S]; row 4q+o = feature o of group nb*16+q*4+a
        o5 = o4.reshape(NB, A_, QD, 4, S)
        o = np.transpose(o5, (0, 2, 1, 4, 3)).reshape(b_core, 4)
        outs.append(o)
    return np.ascontiguousarray(np.concatenate(outs, axis=0).astype(np.float32))


# revision 20
# speedup vs baseline: 1.5519x; 1.3907x over previous
"""Trainium2 Bass kernel for nn_ActionEncoder (moe_routing).

Math (derived from the reference):
  For sample b with t = action_types[b], i0, i1 = action_indecies[b]:
    type 0: out = tanh(W0[:, i0] + b0)
    type 1: out = tanh(W1[:, i0] + W1[:, 64 + i1] + b1)

Routing: the host stable-sorts each core's 65536 samples by type (pure
permutation; inverse-applied to the output).  Device columns then hold
same-type data with unshifted 6-bit keys, so ONE is_equal builds the
one-hot marks for a whole quad:
  - type-0 block: each 512-wide column group packs TWO samples per column
    (rows 0-63 mark i0 of sample A, rows 64-127 mark i0 of sample B); a
    block-diagonal [128, 8] table gathers both samples' 4 features.
  - type-1 block: one sample per column (rows 0-63 mark i0, 64-127 mark
    64+i1); a [128, 4] table (W1.T + b1/2 for both halves) gathers z.
Pipeline per quad (4 groups of 512 cols):
  4 broadcast matmuls spread keys into psum -> 2 psum->sbuf fp16 copies
  (rotating ACT/DVE) -> 1 DVE is_equal (4x mode) -> 4 gather matmuls
  (concurrent col groups) accumulating several quads into one psum bank
  via column-shifted table variants -> one tanh per bank -> fp16 out DMA.
Host reassembles/unsorts to [B, 4] fp32.
"""

import os

import numpy as np

N_CORES = 8
P = 128
S = 512
DVE_COPY_MOD = int(os.environ.get("ACTENC_DVE_COPY_MOD", "4"))

# column-block geometry (per core): type-0 block packs 2 samples/col
G0 = 36                  # type-0 groups: capacity 2*36*512 = 36864 samples
G1 = 68                  # type-1 groups: capacity 68*512 = 34816 samples
G = G0 + G1              # 104 groups = 26 quads
GH = G // 2              # 52 group-pairs per IP half
NQ0, NQ1 = G0 // 4, G1 // 4          # 9 t0 quads, 17 t1 quads
# psum bank packing: t0 banks hold 4 quads (8 rows/group), t1 banks 8 (4 rows)
BANKS0 = [4, 4, 1]       # quads per t0 bank
BANKS1 = [8, 8, 1]       # quads per t1 bank

_NC_CACHE = {}


def _build_nc(b_cols):
    import concourse.mybir as mybir
    from concourse import bacc
    from concourse.tile import TileContext

    f32 = mybir.dt.float32
    i32 = mybir.dt.int32
    f16 = mybir.dt.float16
    eq = mybir.AluOpType.is_equal

    assert b_cols == G * S

    nc = bacc.Bacc("TRN2", target_bir_lowering=False, debug=False)
    idx = nc.dram_tensor("idx", [b_cols, 2], i32, kind="ExternalInput")
    # t0 table: 4 col-shifted block-diag variants [128, 32] at cols 32q
    taba = nc.dram_tensor("taba", [P, 128], f16, kind="ExternalInput")
    # t1 table: 8 col-shifted variants [128, 32] at cols 32q
    tabb = nc.dram_tensor("tabb", [P, 256], f16, kind="ExternalInput")
    selq = nc.dram_tensor("selq", [P, 128 * GH], f16, kind="ExternalInput")
    outa = nc.dram_tensor("outa", [len(BANKS0), 4, 32, S], f16,
                          kind="ExternalOutput")
    outb = nc.dram_tensor("outb", [len(BANKS1), 4, 32, S], f16,
                          kind="ExternalOutput")

    # pair-contiguous DRAM view: [2 halves, GH groups, 2*S] (4KB rows)
    idxp = idx.rearrange("(h g s) c -> h g (s c)", h=2, s=S)

    with TileContext(nc) as tc:
        with tc.tile_pool(name="const", bufs=1) as cpool, \
             tc.tile_pool(name="oh", bufs=6) as ohpool, \
             tc.tile_pool(name="raws", bufs=4) as rpool, \
             tc.tile_pool(name="stage", bufs=2) as spool, \
             tc.tile_pool(name="psb", bufs=3, space="PSUM") as pbpool, \
             tc.tile_pool(name="pszp", bufs=2, space="PSUM") as pzpool:

            # ---- constants ----
            # selector band (host-shipped): view Q[:, 128j : 128j+128]
            # = [e_j x64 | e_{64+j} x64]
            NQb = 128 * GH
            Q = cpool.tile([P, NQb], f16, tag="Q")
            nc.sync.dma_start(out=Q[:, 0:NQb // 2], in_=selq[:, 0:NQb // 2])
            nc.sync.dma_start(out=Q[:, NQb // 2:], in_=selq[:, NQb // 2:])

            # iota2[d] = d mod 64
            ic = cpool.tile([P, 1], i32, tag="ic")
            nc.gpsimd.iota(ic[0:64, :], pattern=[[1, 1]], base=0,
                           channel_multiplier=1)
            nc.gpsimd.iota(ic[64:128, :], pattern=[[1, 1]], base=0,
                           channel_multiplier=1)
            iota2 = cpool.tile([P, 1], f32, tag="iota2")
            nc.vector.tensor_single_scalar(iota2[:], ic[:], 0.0,
                                           mybir.AluOpType.add)

            TA = cpool.tile([P, 128], f16, tag="TA")
            TB = cpool.tile([P, 256], f16, tag="TB")
            nc.sync.dma_start(out=TA[:], in_=taba[:])
            nc.sync.dma_start(out=TB[:], in_=tabb[:])

            # ---- load index pairs; IP[h] rows 0..GH-1 = slot0 keys of the
            #      half's groups, rows 64..64+GH-1 = slot1 keys. Chunked so
            #      early quads start before all input has landed. ----
            IPraw = [cpool.tile([P, 2 * S], i32, tag=f"IPraw{h}",
                                name=f"IPraw{h}") for h in range(2)]
            IP = [cpool.tile([P, S], f16, tag=f"IP{h}", name=f"IP{h}")
                  for h in range(2)]
            for h in range(2):
                # rows GH..63 / 64+GH..127 are never selected but stream
                # through the PE with weight 0 -- zero the whole tile first
                # so stale NaN bit patterns cannot poison psum (0*NaN=NaN)
                nc.vector.memset(IP[h][:], 0)
            # chunks must start 32-partition-aligned
            CHUNKS = [(0, 32), (32, GH - 32)]
            for h in range(2):
                prw3 = IPraw[h][:].rearrange("p (s c) -> p c s", c=2)
                for off, cnt in CHUNKS:
                    gsl = slice(off, off + cnt)
                    for c in range(2):
                        rsl = slice(64 * c + off, 64 * c + off + cnt)
                        nc.sync.dma_start(out=IPraw[h][rsl, :],
                                          in_=idxp[h, gsl])
                        nc.vector.tensor_copy(out=IP[h][rsl, :],
                                              in_=prw3[rsl, c, :])

            # ---- quad schedule: (is_t1, bank, qq(pos in bank), start, stop)
            sched = []
            qt = 0
            for bi, nq in enumerate(BANKS0):
                for qq in range(nq):
                    sched.append((0, bi, qq, qq == 0, qq == nq - 1))
            for bi, nq in enumerate(BANKS1):
                for qq in range(nq):
                    sched.append((1, bi, qq, qq == 0, qq == nq - 1))
            NPI = len(sched)
            assert NPI == NQ0 + NQ1

            def emit_spread(pi):
                raw = rpool.tile([P, 4 * S], f16, tag="raw", name="raw")
                for half in range(2):
                    psb = pbpool.tile([P, 2 * S], f32, tag="psb", name="psb")
                    for i in range(2):
                        g = pi * 4 + half * 2 + i
                        h, j = divmod(g, GH)
                        sel = Q[:, 128 * j:128 * j + 128]
                        nc.tensor.matmul(psb[:, i * S:(i + 1) * S],
                                         lhsT=sel, rhs=IP[h][:],
                                         start=True, stop=True)
                    rsl = slice(half * 2 * S, half * 2 * S + 2 * S)
                    ci = pi * 2 + half
                    if ci % DVE_COPY_MOD == DVE_COPY_MOD - 1:
                        nc.vector.tensor_copy(out=raw[:, rsl], in_=psb[:])
                    else:
                        nc.scalar.copy(raw[:, rsl], psb[:])
                return raw

            psz = None
            raw_next = emit_spread(0)
            for pi in range(NPI):
                is_t1, bi, qq, st, sp = sched[pi]
                if st:
                    psz = pzpool.tile([P, S], f32, tag="psz")
                raw = raw_next
                oh = ohpool.tile([P, 4 * S], f16, tag="oh", name="oh")
                nc.vector.tensor_single_scalar(oh[:], raw[:], iota2[:], eq)
                if pi + 1 < NPI:
                    raw_next = emit_spread(pi + 1)
                tab = TB if is_t1 else TA
                for a in range(4):
                    pz = psz[32 * a:32 * a + 32, :]
                    nc.tensor.matmul(
                        pz, lhsT=tab[:, 32 * qq:32 * qq + 32],
                        rhs=oh[:, a * S:a * S + S],
                        start=st, stop=sp,
                        tile_position=(0, 32 * a),
                        skip_group_check=True)
                if sp:
                    stage = spool.tile([P, S], f16, tag="stage")
                    nc.scalar.activation(
                        out=stage[:], in_=psz[:],
                        func=mybir.ActivationFunctionType.Tanh)
                    outx = outb if is_t1 else outa
                    for a in range(4):
                        nc.sync.dma_start(out=outx[bi, a],
                                          in_=stage[32 * a:32 * a + 32, :])

    nc.compile()
    return nc


def _selq():
    # col block j (j < GH): rows 0-63 = e_j, rows 64-127 = e_{64+j}
    Qm = np.zeros((128, 128 * GH), np.float16)
    k = np.arange(GH)
    f = np.arange(128 * GH)
    top = ((f[None, :] - 128 * k[:, None]) >= 0) & \
          ((f[None, :] - 128 * k[:, None]) < 64)
    bot = ((f[None, :] - 128 * k[:, None] - 64) >= 0) & \
          ((f[None, :] - 128 * k[:, None] - 64) < 64)
    Qm[0:GH, :] = top.astype(np.float16)
    Qm[64:64 + GH, :] = bot.astype(np.float16)
    return Qm


def _tables(W0, b0, W1, b1):
    W0 = np.asarray(W0, np.float32)
    W1 = np.asarray(W1, np.float32)
    b0 = np.asarray(b0, np.float32).reshape(-1)
    b1 = np.asarray(b1, np.float32).reshape(-1)
    T0f = (W0.T + b0).astype(np.float16)          # [64, 4], full bias
    T1f = (W1.T + b1 / 2).astype(np.float16)      # [128, 4], half bias x2
    # t0 variants: block-diagonal two-sample gather, variant qq at cols 32qq:
    #   local col 8qq+o   <- rows 0-63:  T0f   (sample A)
    #   local col 8qq+4+o <- rows 64-127: T0f  (sample B)
    ta = np.zeros((128, 128), np.float16)
    for qq in range(4):
        ta[0:64, 32 * qq + 8 * qq:32 * qq + 8 * qq + 4] = T0f
        ta[64:128, 32 * qq + 8 * qq + 4:32 * qq + 8 * qq + 8] = T0f
    # t1 variants: variant qq at cols 32qq, local col 4qq+o <- T1f
    tb = np.zeros((128, 256), np.float16)
    for qq in range(8):
        tb[:, 32 * qq + 4 * qq:32 * qq + 4 * qq + 4] = T1f
    return ta, tb


def kernel(action_indecies, action_n_obj, action_types, W0, b0, W1, b1,
           **_unused):
    from concourse.bass_utils import run_bass_kernel_spmd

    idx = np.asarray(action_indecies, dtype=np.int32)
    typ = np.asarray(action_types, dtype=np.int32)
    B = idx.shape[0]
    b_core = B // N_CORES
    assert b_core * N_CORES == B
    b_cols = G * S

    ta, tb = _tables(W0, b0, W1, b1)
    selq = _selq()

    if b_cols not in _NC_CACHE:
        _NC_CACHE[b_cols] = _build_nc(b_cols)
    nc = _NC_CACHE[b_cols]

    perms = []
    in_maps = []
    for k in range(N_CORES):
        ik = idx[k * b_core:(k + 1) * b_core]
        tk = typ[k * b_core:(k + 1) * b_core]
        p0 = np.flatnonzero(tk == 0)
        p1 = np.flatnonzero(tk == 1)
        n0, n1 = len(p0), len(p1)
        assert n0 <= 2 * G0 * S and n1 <= G1 * S, (n0, n1)
        e0 = np.zeros(2 * G0 * S, np.int32)
        e0[:n0] = ik[p0, 0]
        v1 = np.zeros((G1 * S, 2), np.int32)
        v1[:n1] = ik[p1, :2]
        colvals = np.concatenate([e0.reshape(-1, 2), v1], axis=0)
        perms.append((p0, p1))
        in_maps.append({"idx": np.ascontiguousarray(colvals),
                        "taba": ta, "tabb": tb, "selq": selq})

    global _last_in_maps
    _last_in_maps = in_maps
    res = run_bass_kernel_spmd(nc, in_maps, core_ids=list(range(N_CORES)))

    outs = []
    for k, r in enumerate(res.results):
        p0, p1 = perms[k]
        n0, n1 = len(p0), len(p1)
        oa = r["outa"]  # [3, 4, 32, S]; band a row 8qq+4u+o, quad = b*4+qq
        ob = r["outb"]  # [3, 4, 32, S]; band a row 4qq+o,   quad = b*8+qq
        # order t0 samples: (quad, a, s, u) -> col = ((quad*4+a)*S+s)*2+u
        a6 = oa.reshape(3, 4, 4, 2, 4, S)          # [b, a, qq, u, o, s]
        a6 = np.transpose(a6, (0, 2, 1, 5, 3, 4))  # [b, qq, a, s, u, o]
        t0vals = a6.reshape(-1, 4)[:2 * G0 * S][:n0]
        b5 = ob.reshape(3, 4, 8, 4, S)             # [b, a, qq, o, s]
        b5 = np.transpose(b5, (0, 2, 1, 4, 3))     # [b, qq, a, s, o]
        t1vals = b5.reshape(-1, 4)[:G1 * S][:n1]
        o = np.empty((b_core, 4), np.float16)
        o[p0] = t0vals
        o[p1] = t1vals
        outs.append(o)
    return np.ascontiguousarray(
        np.concatenate(outs, axis=0).astype(np.float32))
